# revision 1
# baseline (speedup 1.0000x reference)
"""Distributed Trainium2 Bass kernel for a single attention head.

Reference computation (fp32 jax):
    q = queries @ Wq.T + bq        # [B,S,Df]
    k = keys    @ Wk.T + bk
    v = values  @ Wv.T + bv
    attn = softmax((q @ k.T) / sqrt(Df), axis=-1)
    out  = attn @ v                # [B,S,Df]

with B=4, S=4096, D_MODEL=1024, D_FEATURE=64.

Sharding: 8 cores = (batch b in 0..3) x (query-half h in 0..1).
Core c handles batch b=c//2, q rows [h*2048, (h+1)*2048). Each core gets
its q-half plus the FULL keys/values of its batch (no collectives), all
pre-transposed on the host to m-contraction-major layout and converted
to bf16 so matmuls run at full PE rate and DMA bytes are halved.

Kernel structure (per core):
  - inputs arrive i-block-major: [128, nblk * (8 m-chunks * 512 cols)]
    so each 512-column projection block is one contiguous 1MB DMA and
    projection of block i can start as soon as its DMA lands.
  - projections: psum[64, 512] accumulated over 8 m-chunks,
    lhsT = wT chunk [128, 64], rhs = xT chunk [128, 512]; DVE evicts
    psum -> bf16 SBUF with the per-feature bias added.
  - scores are computed TRANSPOSED, flash-style: ST[j, i]
    (lhsT = kT[64, jc*128:...], rhs = qT[64, i-chunk]) so softmax-exp
    input and the attn@v moving operand are both natural layout.
  - attention runs in TWO i-passes of 1024 q-rows each so that the
    scores psum can double-buffer (2x[128,1024] = 4 banks) next to the
    out.T accumulator ([65,1024] = 2 banks) within the 8 PSUM banks.
    Pass A is interleaved with the k/v projections at k-block
    granularity so the ACT engine starts exp-ing ~8us into the kernel.
  - exp is fused with the 1/8 score scale on ACT; output PT is bf16.
  - attn@v accumulates out.T[f, i] with lhsT = v_aug[j, 65] (v in
    natural [j, f] layout + ones column -> row 64 of out.T is the
    softmax denominator for free).
  - finals per pass: evict out.T, PE-transpose 128-row chunks,
    reciprocal of the denominator column, scale, DMA out fp32 rows.
"""

import numpy as np
import ml_dtypes

import concourse.bass as bass
import concourse.mybir as mybir
import concourse.tile as tile
from concourse import bacc
from concourse.bass_utils import run_bass_kernel_spmd
from concourse.masks import make_identity

B = 4
S = 4096
DM = 1024
DF = 64
NCORES = 8
SQ = S // 2          # local q rows per core
MC = DM // 128       # 8 contraction chunks
NI = 512             # moving-operand tile (one PSUM bank of fp32)
JC = S // 128        # 32 key chunks
NBQ = SQ // NI       # 4 q column blocks
NBK = S // NI        # 8 k/v column blocks
IP = SQ // 2         # 1024: i-rows per attention pass
WB = 5 * DF          # per-m-chunk weight columns: [wq|wq|wk|wk|wv]
BF16 = mybir.dt.bfloat16
F32 = mybir.dt.float32
NP_BF16 = ml_dtypes.bfloat16
EXP = mybir.ActivationFunctionType.Exp


def build_kernel(tc):
    nc = tc.nc
    xq = nc.dram_tensor("xq", [128, NBQ * MC * NI], BF16, kind="ExternalInput")
    xk = nc.dram_tensor("xk", [128, NBK * MC * NI], BF16, kind="ExternalInput")
    xv = nc.dram_tensor("xv", [128, NBK * MC * NI], BF16, kind="ExternalInput")
    wT = nc.dram_tensor("wT", [128, MC * WB], BF16, kind="ExternalInput")
    bias = nc.dram_tensor("bias", [128, 3], F32, kind="ExternalInput")
    out = nc.dram_tensor("out", [SQ, DF], F32, kind="ExternalOutput")

    from contextlib import ExitStack

    with ExitStack() as ctx:
        const_pool = ctx.enter_context(tc.tile_pool(name="const", bufs=1))
        xin_pool = ctx.enter_context(tc.tile_pool(name="xin", bufs=9))
        act_pool = ctx.enter_context(tc.tile_pool(name="act", bufs=1))
        pt_pool = ctx.enter_context(tc.tile_pool(name="pt", bufs=4))
        outT_pool = ctx.enter_context(tc.tile_pool(name="outT", bufs=1))
        fin_pool = ctx.enter_context(tc.tile_pool(name="fin", bufs=2))
        # PSUM budget (8 banks): ppsum 2x[64,512] = 2 banks (proj psum,
        # also vtrans/finals scratch), spsum 2x[128,1024] = 4 banks
        # (scores double-buffer), opsum 2 banks (warmup scratch, then the
        # per-pass [65,1024] out.T accumulator).
        ppsum = ctx.enter_context(tc.tile_pool(name="ppsum", bufs=2, space="PSUM"))
        spsum = ctx.enter_context(tc.tile_pool(name="spsum", bufs=2, space="PSUM"))
        opsum = ctx.enter_context(tc.tile_pool(name="opsum", bufs=1, space="PSUM"))

        # ---- constants (DMA'd first) ----
        wT_sb = const_pool.tile([128, MC * WB], BF16, tag="wt")
        nc.sync.dma_start(wT_sb[:], wT[:])
        bias_sb = const_pool.tile([128, 3], F32, tag="bias")
        nc.sync.dma_start(bias_sb[:], bias[:])
        # preload the ACT exp table while DMAs stream
        scratch = const_pool.tile([DF, 1], F32, tag="scratch")
        nc.scalar.activation(scratch[:], bias_sb[0:DF, 0:1], EXP)
        ident = const_pool.tile([128, 128], BF16, tag="ident")
        make_identity(nc, ident[:])
        identf = const_pool.tile([128, 128], F32, tag="identf")
        make_identity(nc, identf[:])

        # ---- PE warm-up: ~7us of dummy matmuls so the HAM clock gate
        # opens (1.2 -> 2.4 GHz) and stays open until the first real
        # matmul's input DMA lands ----
        warm = opsum.tile([DF, 128], F32, tag="po")
        for _ in range(96):
            nc.tensor.matmul(warm[:], ident[:, 0:DF], ident[:], start=True, stop=True)

        # ---- input DMAs, interleaved q first then k/v alternating ----
        def load_block(x_dram, i):
            t = xin_pool.tile([128, MC * NI], BF16, tag="xin")
            nc.sync.dma_start(t[:], x_dram[:, i * MC * NI:(i + 1) * MC * NI])
            return t

        q_tiles = [load_block(xq, i) for i in range(NBQ)]
        kv_tiles = {}
        for i in range(NBK):
            kv_tiles[("k", i)] = load_block(xk, i)
            kv_tiles[("v", i)] = load_block(xv, i)

        # ---- persistent activations ----
        # q/k projections land duplicated in both partition halves so the
        # score matmuls can run pair-wise on independent 64-row PE tiles
        qT_sb = act_pool.tile([128, SQ], BF16, tag="qT")
        kT_sb = act_pool.tile([128, S], BF16, tag="kT")
        vT_sb = act_pool.tile([DF, S], BF16, tag="vT")
        v_sb = act_pool.tile([128, JC * (DF + 1)], BF16, tag="v")  # [128, 32*65]
        nc.gpsimd.memset(v_sb[:], 1.0)  # col DF of every block stays 1.0

        def w_slice(mc_i, which):
            # which: 0 = [wq|wq], 1 = [wk|wk] (128-wide dup), 2 = wv (64)
            o = mc_i * WB + which * 2 * DF
            return wT_sb[:, o:o + (2 * DF if which < 2 else DF)]

        def project_block(x_tile, i, which, dest_sb, bias_col):
            """One 512-column projection block accumulated over 8 m-chunks."""
            rows = 2 * DF if which < 2 else DF
            ps = ppsum.tile([rows, NI], F32, tag="ps")
            for mc_i in range(MC):
                nc.tensor.matmul(
                    ps[:], w_slice(mc_i, which), x_tile[:, mc_i * NI:(mc_i + 1) * NI],
                    start=(mc_i == 0), stop=(mc_i == MC - 1),
                )
            nc.vector.tensor_scalar_add(
                dest_sb[:, i * NI:(i + 1) * NI], ps[:],
                bias_sb[0:rows, bias_col:bias_col + 1])

        # ---- q projection up front ----
        for i in range(NBQ):
            project_block(q_tiles[i], i, 0, qT_sb, 0)

        # pass-B exp results are computed during pass A and parked in SBUF
        ptb_sb = act_pool.tile([128, JC * IP], BF16, tag="ptb")  # 8 MB

        def attn_pair(jc0, poA):
            """Scores + exp for BOTH i-halves of TWO key chunks; the two
            chunks' score matmuls run on independent 64-row PE tiles
            (partitions 0-63 / 64-127 of the duplicated qT/kT), so they
            stream concurrently. attn@v for i-half A follows immediately;
            i-half B's exp output parks in ptb_sb."""
            for ipass in range(2):
                io = ipass * IP
                ss0 = spsum.tile([128, IP], F32, tag="ss", name="ss0")
                ss1 = spsum.tile([128, IP], F32, tag="ss", name="ss1")
                sss = [ss0, ss1]
                for ii in range(IP // NI):
                    for t in range(2):
                        jc = jc0 + t
                        p0 = t * DF
                        nc.tensor.matmul(
                            sss[t][:, ii * NI:(ii + 1) * NI],
                            kT_sb[p0:p0 + DF, jc * 128:(jc + 1) * 128],
                            qT_sb[p0:p0 + DF, io + ii * NI:io + (ii + 1) * NI],
                            start=True, stop=True,
                        )
                for t in range(2):
                    jc = jc0 + t
                    if ipass == 0:
                        pts = pt_pool.tile([128, IP], BF16, tag="pt")
                    else:
                        pts = ptb_sb[:, jc * IP:(jc + 1) * IP]
                    nc.scalar.activation(pts[:], sss[t][:], EXP, scale=0.125)
                    if ipass == 0:
                        for ii in range(IP // NI):
                            nc.tensor.matmul(
                                poA[:, ii * NI:(ii + 1) * NI],
                                v_sb[:, jc * (DF + 1):(jc + 1) * (DF + 1)],
                                pts[:, ii * NI:(ii + 1) * NI],
                                start=(jc == 0), stop=(jc == JC - 1),
                            )

        def finals_chunk(ipass, outT_sb, ob, c):
            pf = ppsum.tile([128, DF + 1], F32, tag="ps")
            nc.tensor.transpose(
                pf[:], outT_sb[:, c * 128:(c + 1) * 128],
                identf[0:DF + 1, 0:DF + 1])
            rcp = fin_pool.tile([128, 1], F32, tag="rcp")
            nc.vector.reciprocal(rcp[:], pf[:, DF:DF + 1])
            nc.vector.tensor_scalar_mul(ob[:, c, :], pf[:, 0:DF], rcp[:])

        def finals_store(ipass, ob):
            # one strided DMA for all 1024 rows of this i-half
            nc.sync.dma_start(
                out[ipass * IP:(ipass + 1) * IP, :].rearrange(
                    "(c p) f -> p c f", p=128),
                ob[:])

        # ---- pass A: k/v projection interleaved with scores/exp for both
        # i-halves + attn@v for i-half A ----
        poA = opsum.tile([DF + 1, IP], F32, tag="po")
        for kb in range(NBK):
            project_block(kv_tiles[("k", kb)], kb, 1, kT_sb, 1)
            project_block(kv_tiles[("v", kb)], kb, 2, vT_sb, 2)
            for jc in range(4 * kb, 4 * kb + 4):
                pv = ppsum.tile([128, DF], BF16, tag="ps")
                nc.tensor.transpose(
                    pv[:], vT_sb[:, jc * 128:(jc + 1) * 128], ident[0:DF, 0:DF])
                nc.vector.tensor_copy(
                    v_sb[:, jc * (DF + 1):jc * (DF + 1) + DF], pv[:])
            for jc0 in range(4 * kb, 4 * kb + 4, 2):
                attn_pair(jc0, poA)

        # ---- pass B: attn@v for i-half B from parked exp outputs; pass A
        # finals are interleaved to fill PE gaps. ----
        outT_A = outT_pool.tile([DF + 1, IP], F32, tag="outT")
        obA = fin_pool.tile([128, IP // 128, DF], F32, tag="ob")
        nc.vector.tensor_copy(outT_A[:], poA[:])
        poB = opsum.tile([DF + 1, IP], F32, tag="po")
        for jc in range(JC):
            for ii in range(IP // NI):
                nc.tensor.matmul(
                    poB[:, ii * NI:(ii + 1) * NI],
                    v_sb[:, jc * (DF + 1):(jc + 1) * (DF + 1)],
                    ptb_sb[:, jc * IP + ii * NI:jc * IP + (ii + 1) * NI],
                    start=(jc == 0), stop=(jc == JC - 1),
                )
            if jc % 4 == 3:
                finals_chunk(0, outT_A, obA, jc // 4)
        finals_store(0, obA)

        outT_B = outT_pool.tile([DF + 1, IP], F32, tag="outT")
        obB = fin_pool.tile([128, IP // 128, DF], F32, tag="ob")
        nc.vector.tensor_copy(outT_B[:], poB[:])
        for c in range(IP // 128):
            finals_chunk(1, outT_B, obB, c)
        finals_store(1, obB)


_COMPILED = None


def get_compiled():
    global _COMPILED
    if _COMPILED is None:
        nc = bacc.Bacc("TRN2", target_bir_lowering=False, debug=False,
                       enable_asserts=False, num_devices=NCORES)
        with tile.TileContext(nc) as tc:
            build_kernel(tc)
        nc.compile()
        _COMPILED = nc
    return _COMPILED


def _to_block_major(xT):
    """[DM, s_len] -> [128, nblk*MC*NI]: 512-col blocks, m-chunk-major inside."""
    s_len = xT.shape[1]
    nblk = s_len // NI
    # (mc, p, blk, s) -> (p, blk, mc, s)
    return np.ascontiguousarray(
        xT.reshape(MC, 128, nblk, NI).transpose(1, 2, 0, 3).reshape(128, nblk * MC * NI))


def make_in_maps(queries, keys, values, Wq, bq, Wk, bk, Wv, bv):
    queries = np.asarray(queries, dtype=np.float32)
    keys = np.asarray(keys, dtype=np.float32)
    values = np.asarray(values, dtype=np.float32)
    WqT, WkT, WvT = np.asarray(Wq).T, np.asarray(Wk).T, np.asarray(Wv).T
    wT_full = np.concatenate([WqT, WqT, WkT, WkT, WvT], axis=1)  # [DM, 320]
    wT_host = np.ascontiguousarray(
        wT_full.reshape(MC, 128, WB).transpose(1, 0, 2).reshape(128, MC * WB)
    ).astype(NP_BF16)
    bias64 = np.stack(
        [np.asarray(bq), np.asarray(bk), np.asarray(bv)], axis=1
    ).astype(np.float32)
    bias_host = np.concatenate([bias64, bias64], axis=0)  # [128, 3]

    in_maps = []
    for c in range(NCORES):
        b, h = c // 2, c % 2
        in_maps.append({
            "xq": _to_block_major(queries[b, h * SQ:(h + 1) * SQ, :].T).astype(NP_BF16),
            "xk": _to_block_major(keys[b].T).astype(NP_BF16),
            "xv": _to_block_major(values[b].T).astype(NP_BF16),
            "wT": wT_host, "bias": bias_host,
        })
    return in_maps


def assemble(results):
    out = np.zeros((B, S, DF), dtype=np.float32)
    for c in range(NCORES):
        b, h = c // 2, c % 2
        out[b, h * SQ:(h + 1) * SQ, :] = results[c]["out"]
    return out


def kernel(**inputs):
    nc = get_compiled()
    in_maps = make_in_maps(**inputs)
    res = run_bass_kernel_spmd(nc, in_maps, core_ids=list(range(NCORES)))
    return assemble(res.results)



# revision 6
# speedup vs baseline: 1.0820x; 1.0820x over previous
"""Distributed Trainium2 Bass kernel for a single attention head.

Reference computation (fp32 jax):
    q = queries @ Wq.T + bq        # [B,S,Df]
    k = keys    @ Wk.T + bk
    v = values  @ Wv.T + bv
    attn = softmax((q @ k.T) / sqrt(Df), axis=-1)
    out  = attn @ v                # [B,S,Df]

with B=4, S=4096, D_MODEL=1024, D_FEATURE=64.

Sharding: 8 cores = (batch b in 0..3) x (query-half h in 0..1).
Core c handles batch b=c//2, q rows [h*2048, (h+1)*2048). Each core gets
its q-half plus the FULL keys/values of its batch (no collectives), all
pre-transposed on the host to m-contraction-major layout and bf16.

Kernel structure (per core), ACT-throughput-oriented redesign:
  - k/v projections are COL-PACKED: blocks (2t, 2t+1) project
    concurrently into psum partitions 0-63 / 64-127 (tile_position
    (0,0) / (0,64)), halving projection PE time. kT lands SPLIT:
    partitions 0-63 hold even-block features, 64-127 odd-block --
    exactly the layout the score pairs need, so k needs NO dup.
  - q projection uses [wq|wq] dup'd weights (M=128) because both score
    row-groups stream the same q columns.
  - scores for chunk-pair (t,c) run as 2 concurrent row-group matmuls
    (K=64) into ONE [128, 1024] psum tile: cols 0:512 = even chunk,
    512:1024 = odd chunk, for one 512-wide i-slice. ONE exp per tile
    (ACT is the kernel bottleneck: 64 tiles x ~1.15us = 73us; wider
    tiles amortize the +352cyc instruction overhead).
  - attn@v is ROW-SPLIT 2-concurrent: j-rows 0-63 -> psumA, 64-127 ->
    psumB (tile (0,0)/(64,0)), each [65, 512] in its own bank,
    accumulating all 32 chunks; v_aug ones-column gives both partial
    softmax denominators; final outT = psumA + psumB on DVE.
  - i is processed in 4 passes of 512 q-rows. kT/qT/v_sb stay resident;
    exp outputs park in a 52-deep SBUF ring so ACT never stalls while
    attn passes drain serially (psum A/B single-buffered).
  - PSUM: ss 2x[128,1024]f32 (4 banks) + psumA/B [65,512]f32 (2) +
    work pool for proj/transpose/finals (2) = 8 banks exactly.
  - Emission order is pipeline-simulated: scores tile n executes at
    ~ACT pace (ss ring bufs=2), so attn rounds/projections/finals are
    interleaved between score emissions to keep the in-order PE queue
    from head-of-line blocking.
"""

import numpy as np
import ml_dtypes

import concourse.bass as bass
import concourse.mybir as mybir
import concourse.tile as tile
from concourse import bacc
from concourse.bass_utils import run_bass_kernel_spmd
from concourse.masks import make_identity

B = 4
S = 4096
DM = 1024
DF = 64
NCORES = 8
SQ = S // 2          # local q rows per core
MC = DM // 128       # 8 contraction chunks
NI = 512             # proj block / i-slice width
NBQ = SQ // NI       # 4 q blocks
NBK = S // NI        # 8 k/v blocks
NT = NBK // 2        # 4 k/v block-pairs
JC = S // 128        # 32 key chunks of 128
IP = NI              # i rows per attention pass (512)
NP = SQ // IP        # 4 passes
WB = 4 * DF          # per-m-chunk weight cols: [wq|wq|wk|wv] = 256
BF16 = mybir.dt.bfloat16
F32 = mybir.dt.float32
NP_BF16 = ml_dtypes.bfloat16
EXP = mybir.ActivationFunctionType.Exp

# input DMA order (1MB blocks): early k pairs + q blocks feed scores
# ASAP; v trails since attn drains from the parked-exp ring late.
DMA_ORDER = [
    ("q", 0), ("q", 1), ("k", 0), ("k", 1), ("q", 2), ("q", 3),
    ("k", 2), ("k", 3), ("v", 0), ("v", 1), ("k", 4), ("k", 5),
    ("v", 2), ("v", 3), ("k", 6), ("k", 7), ("v", 4), ("v", 5),
    ("v", 6), ("v", 7),
]


def build_kernel(tc):
    nc = tc.nc
    xq = nc.dram_tensor("xq", [128, NBQ * MC * NI], BF16, kind="ExternalInput")
    xk = nc.dram_tensor("xk", [128, NBK * MC * NI], BF16, kind="ExternalInput")
    xv = nc.dram_tensor("xv", [128, NBK * MC * NI], BF16, kind="ExternalInput")
    wT = nc.dram_tensor("wT", [128, MC * WB], BF16, kind="ExternalInput")
    bias = nc.dram_tensor("bias", [128, 3], F32, kind="ExternalInput")
    out = nc.dram_tensor("out", [SQ, DF], F32, kind="ExternalOutput")

    from contextlib import ExitStack

    with ExitStack() as ctx:
        const_pool = ctx.enter_context(tc.tile_pool(name="const", bufs=1))
        xin_pool = ctx.enter_context(tc.tile_pool(name="xin", bufs=6))
        act_pool = ctx.enter_context(tc.tile_pool(name="act", bufs=1))
        vtmp_pool = ctx.enter_context(tc.tile_pool(name="vtmp", bufs=2))
        pt_pool = ctx.enter_context(tc.tile_pool(name="pt", bufs=52))
        outT_pool = ctx.enter_context(tc.tile_pool(name="outT", bufs=2))
        ob_pool = ctx.enter_context(tc.tile_pool(name="ob", bufs=2))
        rcp_pool = ctx.enter_context(tc.tile_pool(name="rcp", bufs=4))
        # PSUM budget (8 banks): ss 2x[128,1024]f32 = 4, psumA/B
        # [65,512]f32 = 1+1, work pool (proj/vtrans/finals) 2x2KB = 2.
        spsum = ctx.enter_context(tc.tile_pool(name="spsum", bufs=2, space="PSUM"))
        apsA = ctx.enter_context(tc.tile_pool(name="apsA", bufs=1, space="PSUM"))
        apsB = ctx.enter_context(tc.tile_pool(name="apsB", bufs=1, space="PSUM"))
        wpsum = ctx.enter_context(tc.tile_pool(name="wpsum", bufs=2, space="PSUM"))

        # ---- constants ----
        wT_sb = const_pool.tile([128, MC * WB], BF16, tag="wt")
        nc.sync.dma_start(wT_sb[:], wT[:])
        bias_sb = const_pool.tile([128, 3], F32, tag="bias")
        nc.sync.dma_start(bias_sb[:], bias[:])
        # preload the ACT exp table while DMAs stream
        scratch = const_pool.tile([DF, 1], F32, tag="scratch")
        nc.scalar.activation(scratch[:], bias_sb[0:DF, 0:1], EXP)
        ident = const_pool.tile([128, 128], BF16, tag="ident")
        make_identity(nc, ident[:])
        identf = const_pool.tile([128, 128], F32, tag="identf")
        make_identity(nc, identf[:])

        # ---- PE warm-up: open the HAM clock gate before real work ----
        warm = wpsum.tile([DF, 128], F32, tag="ps")
        for _ in range(96):
            nc.tensor.matmul(warm[:], ident[:, 0:DF], ident[:], start=True, stop=True)

        # ---- input DMAs in pipeline order ----
        xmap = {"q": xq, "k": xk, "v": xv}
        tiles = {}
        for kind, i in DMA_ORDER:
            t = xin_pool.tile([128, MC * NI], BF16, tag="xin")
            nc.sync.dma_start(t[:], xmap[kind][:, i * MC * NI:(i + 1) * MC * NI])
            tiles[(kind, i)] = t

        # ---- persistent activations ----
        qT_sb = act_pool.tile([128, SQ], BF16, tag="qT")   # dup'd halves
        kT_sb = act_pool.tile([128, SQ], BF16, tag="kT")   # split even/odd blocks
        v_sb = act_pool.tile([128, JC * (DF + 1)], BF16, tag="v")
        nc.gpsimd.memset(v_sb[:], 1.0)  # col DF of every chunk stays 1.0

        def qproj(p):
            ps = wpsum.tile([128, NI], F32, tag="ps")
            x = tiles[("q", p)]
            for mc in range(MC):
                nc.tensor.matmul(
                    ps[:], wT_sb[:, mc * WB:mc * WB + 128],
                    x[:, mc * NI:(mc + 1) * NI],
                    start=(mc == 0), stop=(mc == MC - 1))
            nc.vector.tensor_scalar_add(
                qT_sb[:, p * NI:(p + 1) * NI], ps[:], bias_sb[0:128, 0:1])

        def kvproj(t, which):
            """Col-packed pair (2t, 2t+1): even block -> psum[0:64],
            odd -> psum[64:128] (tile_position (0,64) auto-derived)."""
            kind = "k" if which == 1 else "v"
            wofs = 128 + (which - 1) * DF
            # separate banks for the two streams: even block -> psE[0:64]
            # (array cols 0-63), odd -> psO[64:128] (tile_position (0,64)).
            psE = wpsum.tile([128, NI], F32, tag="ps", name="psE")
            psO = wpsum.tile([128, NI], F32, tag="ps", name="psO")
            xe, xo = tiles[(kind, 2 * t)], tiles[(kind, 2 * t + 1)]
            for mc in range(MC):
                w = wT_sb[:, mc * WB + wofs:mc * WB + wofs + DF]
                nc.tensor.matmul(
                    psE[0:DF, :], w, xe[:, mc * NI:(mc + 1) * NI],
                    start=(mc == 0), stop=(mc == MC - 1))
                nc.tensor.matmul(
                    psO[DF:128, :], w, xo[:, mc * NI:(mc + 1) * NI],
                    start=(mc == 0), stop=(mc == MC - 1))
            dst = None
            if which == 1:
                de = kT_sb[0:DF, t * NI:(t + 1) * NI]
                do = kT_sb[DF:128, t * NI:(t + 1) * NI]
                bcol = 1
            else:
                dst = vtmp_pool.tile([128, NI], BF16, tag="vtmp", name="vtmp")
                de, do = dst[0:DF, :], dst[DF:128, :]
                bcol = 2
            nc.vector.tensor_scalar_add(
                de, psE[0:DF, :], bias_sb[0:DF, bcol:bcol + 1])
            nc.vector.tensor_scalar_add(
                do, psO[DF:128, :], bias_sb[DF:128, bcol:bcol + 1])
            return dst

        def vtrans(t, vtmp):
            """[128,128] PE transposes: each yields features for chunk
            8t+c (cols 0:64) and 8t+4+c (cols 64:128) of v_sb."""
            for c in range(4):
                pv = wpsum.tile([128, 128], BF16, tag="ps")
                nc.tensor.transpose(
                    pv[:], vtmp[:, c * 128:(c + 1) * 128], ident[:])
                je, jo = 8 * t + c, 8 * t + 4 + c
                nc.vector.tensor_copy(
                    v_sb[:, je * (DF + 1):je * (DF + 1) + DF], pv[:, 0:DF])
                nc.vector.tensor_copy(
                    v_sb[:, jo * (DF + 1):jo * (DF + 1) + DF], pv[:, DF:128])

        pts = {}

        def sc(n):
            """Scores + exp for tile n: t=n//16, p=(n//4)%4, c=n%4.
            Even chunk (8t+c) -> ss[:,0:512], odd (8t+4+c) -> ss[:,512:]."""
            t, p, c = n // 16, (n // 4) % 4, n % 4
            ss = spsum.tile([128, 2 * IP], F32, tag="ss")
            col = t * NI + c * 128
            nc.tensor.matmul(
                ss[:, 0:IP], kT_sb[0:DF, col:col + 128],
                qT_sb[0:DF, p * IP:(p + 1) * IP], start=True, stop=True)
            nc.tensor.matmul(
                ss[:, IP:2 * IP], kT_sb[DF:128, col:col + 128],
                qT_sb[DF:128, p * IP:(p + 1) * IP], start=True, stop=True)
            pt = pt_pool.tile([128, 2 * IP], BF16, tag="pt")
            nc.scalar.activation(pt[:], ss[:], EXP, scale=0.125)
            pts[n] = pt

        cur = {}

        def pass_begin():
            cur["A"] = apsA.tile([DF + 1, IP], F32, tag="pa", name="psA")
            cur["B"] = apsB.tile([DF + 1, IP], F32, tag="pb", name="psB")

        def at(p, t):
            """attn@v rounds for pass p, block-pair t: row-split
            concurrent accumulation into psumA (j 0:64) / psumB."""
            psA, psB = cur["A"], cur["B"]
            for c in range(4):
                pt = pts[t * 16 + p * 4 + c]
                je, jo = 8 * t + c, 8 * t + 4 + c
                first = (t == 0 and c == 0)
                last = (t == NT - 1 and c == 3)
                nc.tensor.matmul(
                    psA[:], v_sb[0:DF, je * (DF + 1):(je + 1) * (DF + 1)],
                    pt[0:DF, 0:IP], start=first, stop=False)
                nc.tensor.matmul(
                    psB[:], v_sb[DF:128, je * (DF + 1):(je + 1) * (DF + 1)],
                    pt[DF:128, 0:IP], start=first, stop=False)
                nc.tensor.matmul(
                    psA[:], v_sb[0:DF, jo * (DF + 1):(jo + 1) * (DF + 1)],
                    pt[0:DF, IP:2 * IP], start=False, stop=last)
                nc.tensor.matmul(
                    psB[:], v_sb[DF:128, jo * (DF + 1):(jo + 1) * (DF + 1)],
                    pt[DF:128, IP:2 * IP], start=False, stop=last)

        outTs = {}

        def ev(p):
            oT = outT_pool.tile([DF + 1, IP], F32, tag="ot")
            # DVE may read only one PSUM operand per instruction
            nc.vector.tensor_copy(oT[:], cur["A"][:])
            nc.vector.tensor_add(oT[:], oT[:], cur["B"][:])
            outTs[p] = oT

        def fin(p):
            oT = outTs[p]
            ob = ob_pool.tile([128, IP // 128, DF], F32, tag="ob")
            for c in range(IP // 128):
                pf = wpsum.tile([128, DF + 1], F32, tag="ps")
                nc.tensor.transpose(
                    pf[:], oT[:, c * 128:(c + 1) * 128],
                    identf[0:DF + 1, 0:DF + 1])
                rcp = rcp_pool.tile([128, 1], F32, tag="rcp")
                nc.vector.reciprocal(rcp[:], pf[:, DF:DF + 1])
                nc.vector.tensor_scalar_mul(ob[:, c, :], pf[:, 0:DF], rcp[:])
            nc.sync.dma_start(
                out[p * IP:(p + 1) * IP, :].rearrange("(c p) f -> p c f", p=128),
                ob[:])

        # ---- emission schedule (see module docstring) ----
        qproj(0); qproj(1)
        kvproj(0, 1)
        for n in range(0, 8):
            sc(n)
        qproj(2); qproj(3)
        for n in range(8, 13):
            sc(n)
        vt0 = kvproj(0, 2); vtrans(0, vt0)
        pass_begin()
        at(0, 0)
        for n in range(13, 16):
            sc(n)
        kvproj(1, 1)
        for n in range(16, 22):
            sc(n)
        vt1 = kvproj(1, 2); vtrans(1, vt1)
        for n in range(22, 24):
            sc(n)
        at(0, 1)
        for n in range(24, 32):
            sc(n)
        kvproj(2, 1)
        for n in range(32, 34):
            sc(n)
        vt2 = kvproj(2, 2); vtrans(2, vt2)
        for n in range(34, 38):
            sc(n)
        at(0, 2)
        for n in range(38, 42):
            sc(n)
        vt3 = kvproj(3, 2); vtrans(3, vt3)
        for n in range(42, 44):
            sc(n)
        kvproj(3, 1)
        for n in range(44, 53):
            sc(n)
        at(0, 3); ev(0)
        sc(53)
        pass_begin()
        at(1, 0)
        sc(54)
        at(1, 1)
        sc(55)
        at(1, 2)
        sc(56)
        at(1, 3); ev(1)
        sc(57)
        pass_begin()
        at(2, 0)
        fin(0)
        sc(58)
        at(2, 1)
        sc(59)
        at(2, 2)
        sc(60)
        at(2, 3); ev(2)
        sc(61)
        pass_begin()
        at(3, 0)
        fin(1)
        sc(62)
        at(3, 1)
        sc(63)
        at(3, 2)
        at(3, 3); ev(3)
        fin(2); fin(3)


_COMPILED = None


def get_compiled():
    global _COMPILED
    if _COMPILED is None:
        nc = bacc.Bacc("TRN2", target_bir_lowering=False, debug=False,
                       enable_asserts=False, num_devices=NCORES)
        with tile.TileContext(nc) as tc:
            build_kernel(tc)
        nc.compile()
        _COMPILED = nc
    return _COMPILED


def _to_block_major(xT):
    """[DM, s_len] -> [128, nblk*MC*NI]: 512-col blocks, m-chunk-major inside."""
    s_len = xT.shape[1]
    nblk = s_len // NI
    return np.ascontiguousarray(
        xT.reshape(MC, 128, nblk, NI).transpose(1, 2, 0, 3).reshape(128, nblk * MC * NI))


def make_in_maps(queries, keys, values, Wq, bq, Wk, bk, Wv, bv):
    queries = np.asarray(queries, dtype=np.float32)
    keys = np.asarray(keys, dtype=np.float32)
    values = np.asarray(values, dtype=np.float32)
    WqT, WkT, WvT = np.asarray(Wq).T, np.asarray(Wk).T, np.asarray(Wv).T
    wT_full = np.concatenate([WqT, WqT, WkT, WvT], axis=1)  # [DM, 256]
    wT_host = np.ascontiguousarray(
        wT_full.reshape(MC, 128, WB).transpose(1, 0, 2).reshape(128, MC * WB)
    ).astype(NP_BF16)
    bias64 = np.stack(
        [np.asarray(bq), np.asarray(bk), np.asarray(bv)], axis=1
    ).astype(np.float32)
    bias_host = np.concatenate([bias64, bias64], axis=0)  # [128, 3]

    in_maps = []
    for c in range(NCORES):
        b, h = c // 2, c % 2
        in_maps.append({
            "xq": _to_block_major(queries[b, h * SQ:(h + 1) * SQ, :].T).astype(NP_BF16),
            "xk": _to_block_major(keys[b].T).astype(NP_BF16),
            "xv": _to_block_major(values[b].T).astype(NP_BF16),
            "wT": wT_host, "bias": bias_host,
        })
    return in_maps


def assemble(results):
    out = np.zeros((B, S, DF), dtype=np.float32)
    for c in range(NCORES):
        b, h = c // 2, c % 2
        out[b, h * SQ:(h + 1) * SQ, :] = results[c]["out"]
    return out


def kernel(**inputs):
    nc = get_compiled()
    in_maps = make_in_maps(**inputs)
    res = run_bass_kernel_spmd(nc, in_maps, core_ids=list(range(NCORES)))
    return assemble(res.results)


# revision 9
# speedup vs baseline: 1.1255x; 1.0402x over previous
"""Distributed Trainium2 Bass kernel for a single attention head.

Reference computation (fp32 jax):
    q = queries @ Wq.T + bq        # [B,S,Df]
    k = keys    @ Wk.T + bk
    v = values  @ Wv.T + bv
    attn = softmax((q @ k.T) / sqrt(Df), axis=-1)
    out  = attn @ v                # [B,S,Df]

with B=4, S=4096, D_MODEL=1024, D_FEATURE=64.

Sharding: 8 cores = (batch b in 0..3) x (query-half h in 0..1).
Core c handles batch b=c//2, q rows [h*2048, (h+1)*2048). Each core gets
its q-half plus the FULL keys/values of its batch (no collectives), all
pre-transposed on the host to m-contraction-major layout and bf16.

Kernel structure (per core), ACT-throughput-oriented redesign:
  - k/v projections are COL-PACKED: blocks (2t, 2t+1) project
    concurrently into psum partitions 0-63 / 64-127 (tile_position
    (0,0) / (0,64)), halving projection PE time. kT lands SPLIT:
    partitions 0-63 hold even-block features, 64-127 odd-block --
    exactly the layout the score pairs need, so k needs NO dup.
  - q projection uses [wq|wq] dup'd weights (M=128) because both score
    row-groups stream the same q columns.
  - scores for chunk-pair (t,c) run as 2 concurrent row-group matmuls
    (K=64) into ONE [128, 1024] psum tile: cols 0:512 = even chunk,
    512:1024 = odd chunk, for one 512-wide i-slice. ONE exp per tile
    (ACT is the kernel bottleneck: 64 tiles x ~1.15us = 73us; wider
    tiles amortize the +352cyc instruction overhead).
  - attn@v is ROW-SPLIT 2-concurrent: j-rows 0-63 -> psumA, 64-127 ->
    psumB (tile (0,0)/(64,0)), each [65, 512] in its own bank,
    accumulating all 32 chunks; v_aug ones-column gives both partial
    softmax denominators; final outT = psumA + psumB on DVE.
  - i is processed in 4 passes of 512 q-rows. kT/qT/v_sb stay resident;
    exp outputs park in a 52-deep SBUF ring so ACT never stalls while
    attn passes drain serially (psum A/B single-buffered).
  - PSUM: ss 2x[128,1024]f32 (4 banks) + psumA/B [65,512]f32 (2) +
    work pool for proj/transpose/finals (2) = 8 banks exactly.
  - Emission order is pipeline-simulated: scores tile n executes at
    ~ACT pace (ss ring bufs=2), so attn rounds/projections/finals are
    interleaved between score emissions to keep the in-order PE queue
    from head-of-line blocking.
"""

import numpy as np
import ml_dtypes

import concourse.bass as bass
import concourse.mybir as mybir
import concourse.tile as tile
from concourse import bacc
from concourse.bass_utils import run_bass_kernel_spmd
from concourse.masks import make_identity

B = 4
S = 4096
DM = 1024
DF = 64
NCORES = 8
SQ = S // 2          # local q rows per core
MC = DM // 128       # 8 contraction chunks
NI = 512             # proj block / i-slice width
NBQ = SQ // NI       # 4 q blocks
NBK = S // NI        # 8 k/v blocks
NT = NBK // 2        # 4 k/v block-pairs
JC = S // 128        # 32 key chunks of 128
IP = NI              # i rows per attention pass (512)
NP = SQ // IP        # 4 passes
WB = 4 * DF          # per-m-chunk weight cols: [wq|wq|wk|wv] = 256
BF16 = mybir.dt.bfloat16
F32 = mybir.dt.float32
NP_BF16 = ml_dtypes.bfloat16
EXP = mybir.ActivationFunctionType.Exp

# input DMA order (1MB blocks): k pairs + q blocks first (they gate the
# ACT exp stream, the kernel bottleneck); v trails since attn drains
# from the parked-exp ring late.
DMA_ORDER = [
    ("k", 0), ("k", 1), ("q", 0), ("q", 1), ("k", 2), ("k", 3),
    ("q", 2), ("q", 3), ("k", 4), ("k", 5), ("v", 0), ("v", 1),
    ("k", 6), ("k", 7), ("v", 2), ("v", 3), ("v", 4), ("v", 5),
    ("v", 6), ("v", 7),
]

# score-tile emission order: t-major while DMA supplies k pairs, then
# pass-major so the serial attn pass chain tracks the exp stream tail.
TP_ORDER = [
    (0, 0), (0, 1), (0, 2), (0, 3), (1, 0), (1, 1), (1, 2), (1, 3),
    (2, 0), (3, 0), (2, 1), (3, 1), (2, 2), (3, 2), (2, 3), (3, 3),
]


def build_kernel(tc):
    nc = tc.nc
    xq = nc.dram_tensor("xq", [128, NBQ * MC * NI], BF16, kind="ExternalInput")
    xk = nc.dram_tensor("xk", [128, NBK * MC * NI], BF16, kind="ExternalInput")
    xv = nc.dram_tensor("xv", [128, NBK * MC * NI], BF16, kind="ExternalInput")
    wT = nc.dram_tensor("wT", [128, MC * WB], BF16, kind="ExternalInput")
    bias = nc.dram_tensor("bias", [128, 3], F32, kind="ExternalInput")
    out = nc.dram_tensor("out", [SQ, DF], F32, kind="ExternalOutput")

    from contextlib import ExitStack

    with ExitStack() as ctx:
        const_pool = ctx.enter_context(tc.tile_pool(name="const", bufs=1))
        xin_pool = ctx.enter_context(tc.tile_pool(name="xin", bufs=6))
        act_pool = ctx.enter_context(tc.tile_pool(name="act", bufs=1))
        vtmp_pool = ctx.enter_context(tc.tile_pool(name="vtmp", bufs=2))
        pt_pool = ctx.enter_context(tc.tile_pool(name="pt", bufs=52))
        outT_pool = ctx.enter_context(tc.tile_pool(name="outT", bufs=2))
        ob_pool = ctx.enter_context(tc.tile_pool(name="ob", bufs=2))
        rcp_pool = ctx.enter_context(tc.tile_pool(name="rcp", bufs=4))
        # PSUM budget (8 banks): ss 2x[128,1024]f32 = 4, psumA/B
        # [65,512]f32 = 1+1, work pool (proj/vtrans/finals) 2x2KB = 2.
        spsum = ctx.enter_context(tc.tile_pool(name="spsum", bufs=2, space="PSUM"))
        apsA = ctx.enter_context(tc.tile_pool(name="apsA", bufs=1, space="PSUM"))
        apsB = ctx.enter_context(tc.tile_pool(name="apsB", bufs=1, space="PSUM"))
        wpsum = ctx.enter_context(tc.tile_pool(name="wpsum", bufs=2, space="PSUM"))

        # ---- constants ----
        wT_sb = const_pool.tile([128, MC * WB], BF16, tag="wt")
        nc.sync.dma_start(wT_sb[:], wT[:])
        bias_sb = const_pool.tile([128, 3], F32, tag="bias")
        nc.sync.dma_start(bias_sb[:], bias[:])
        # preload the ACT exp table while DMAs stream
        scratch = const_pool.tile([DF, 1], F32, tag="scratch")
        nc.scalar.activation(scratch[:], bias_sb[0:DF, 0:1], EXP)
        ident = const_pool.tile([128, 128], BF16, tag="ident")
        make_identity(nc, ident[:])
        identf = const_pool.tile([128, 128], F32, tag="identf")
        make_identity(nc, identf[:])

        # ---- PE warm-up: open the HAM clock gate before real work ----
        warm = wpsum.tile([DF, 128], F32, tag="ps")
        for _ in range(96):
            nc.tensor.matmul(warm[:], ident[:, 0:DF], ident[:], start=True, stop=True)

        # ---- input DMAs in pipeline order ----
        xmap = {"q": xq, "k": xk, "v": xv}
        tiles = {}
        for kind, i in DMA_ORDER:
            t = xin_pool.tile([128, MC * NI], BF16, tag="xin")
            nc.sync.dma_start(t[:], xmap[kind][:, i * MC * NI:(i + 1) * MC * NI])
            tiles[(kind, i)] = t

        # ---- persistent activations ----
        qT_sb = act_pool.tile([128, SQ], BF16, tag="qT")   # dup'd halves
        kT_sb = act_pool.tile([128, SQ], BF16, tag="kT")   # split even/odd blocks
        v_sb = act_pool.tile([128, JC * (DF + 1)], BF16, tag="v")
        nc.gpsimd.memset(v_sb[:], 1.0)  # col DF of every chunk stays 1.0

        def qproj(p):
            ps = wpsum.tile([128, NI], F32, tag="ps")
            x = tiles[("q", p)]
            for mc in range(MC):
                nc.tensor.matmul(
                    ps[:], wT_sb[:, mc * WB:mc * WB + 128],
                    x[:, mc * NI:(mc + 1) * NI],
                    start=(mc == 0), stop=(mc == MC - 1))
            nc.vector.tensor_scalar_add(
                qT_sb[:, p * NI:(p + 1) * NI], ps[:], bias_sb[0:128, 0:1])

        def kvproj(t, which):
            """Col-packed pair (2t, 2t+1): even block -> psum[0:64],
            odd -> psum[64:128] (tile_position (0,64) auto-derived)."""
            kind = "k" if which == 1 else "v"
            wofs = 128 + (which - 1) * DF
            # separate banks for the two streams: even block -> psE[0:64]
            # (array cols 0-63), odd -> psO[64:128] (tile_position (0,64)).
            psE = wpsum.tile([128, NI], F32, tag="ps", name="psE")
            psO = wpsum.tile([128, NI], F32, tag="ps", name="psO")
            xe, xo = tiles[(kind, 2 * t)], tiles[(kind, 2 * t + 1)]
            for mc in range(MC):
                w = wT_sb[:, mc * WB + wofs:mc * WB + wofs + DF]
                nc.tensor.matmul(
                    psE[0:DF, :], w, xe[:, mc * NI:(mc + 1) * NI],
                    start=(mc == 0), stop=(mc == MC - 1))
                nc.tensor.matmul(
                    psO[DF:128, :], w, xo[:, mc * NI:(mc + 1) * NI],
                    start=(mc == 0), stop=(mc == MC - 1))
            dst = None
            if which == 1:
                de = kT_sb[0:DF, t * NI:(t + 1) * NI]
                do = kT_sb[DF:128, t * NI:(t + 1) * NI]
                bcol = 1
            else:
                dst = vtmp_pool.tile([128, NI], BF16, tag="vtmp", name="vtmp")
                de, do = dst[0:DF, :], dst[DF:128, :]
                bcol = 2
            nc.vector.tensor_scalar_add(
                de, psE[0:DF, :], bias_sb[0:DF, bcol:bcol + 1])
            nc.vector.tensor_scalar_add(
                do, psO[DF:128, :], bias_sb[DF:128, bcol:bcol + 1])
            return dst

        def vtrans(t, vtmp):
            """[128,128] PE transposes: each yields features for chunk
            8t+c (cols 0:64) and 8t+4+c (cols 64:128) of v_sb."""
            for c in range(4):
                pv = wpsum.tile([128, 128], BF16, tag="ps")
                nc.tensor.transpose(
                    pv[:], vtmp[:, c * 128:(c + 1) * 128], ident[:])
                je, jo = 8 * t + c, 8 * t + 4 + c
                nc.vector.tensor_copy(
                    v_sb[:, je * (DF + 1):je * (DF + 1) + DF], pv[:, 0:DF])
                nc.vector.tensor_copy(
                    v_sb[:, jo * (DF + 1):jo * (DF + 1) + DF], pv[:, DF:128])

        pts = {}

        def sc(t, p, c):
            """Scores + exp for chunk-pair (t,c), i-slice p. Even chunk
            (8t+c) -> ss[:,0:512], odd (8t+4+c) -> ss[:,512:1024]."""
            ss = spsum.tile([128, 2 * IP], F32, tag="ss")
            col = t * NI + c * 128
            nc.tensor.matmul(
                ss[:, 0:IP], kT_sb[0:DF, col:col + 128],
                qT_sb[0:DF, p * IP:(p + 1) * IP], start=True, stop=True)
            nc.tensor.matmul(
                ss[:, IP:2 * IP], kT_sb[DF:128, col:col + 128],
                qT_sb[DF:128, p * IP:(p + 1) * IP], start=True, stop=True)
            pt = pt_pool.tile([128, 2 * IP], BF16, tag="pt")
            nc.scalar.activation(pt[:], ss[:], EXP, scale=0.125)
            pts[(t, p, c)] = pt

        cur = {}

        def pass_begin():
            cur["A"] = apsA.tile([DF + 1, IP], F32, tag="pa", name="psA")
            cur["B"] = apsB.tile([DF + 1, IP], F32, tag="pb", name="psB")

        def at_piece(p, t, c):
            """One attn@v piece: row-split concurrent accumulation of
            chunks 8t+c (even) and 8t+4+c (odd) into psumA/psumB."""
            psA, psB = cur["A"], cur["B"]
            pt = pts[(t, p, c)]
            je, jo = 8 * t + c, 8 * t + 4 + c
            first = (t == 0 and c == 0)
            last = (t == NT - 1 and c == 3)
            nc.tensor.matmul(
                psA[:], v_sb[0:DF, je * (DF + 1):(je + 1) * (DF + 1)],
                pt[0:DF, 0:IP], start=first, stop=False)
            nc.tensor.matmul(
                psB[:], v_sb[DF:128, je * (DF + 1):(je + 1) * (DF + 1)],
                pt[DF:128, 0:IP], start=first, stop=False)
            nc.tensor.matmul(
                psA[:], v_sb[0:DF, jo * (DF + 1):(jo + 1) * (DF + 1)],
                pt[0:DF, IP:2 * IP], start=False, stop=last)
            nc.tensor.matmul(
                psB[:], v_sb[DF:128, jo * (DF + 1):(jo + 1) * (DF + 1)],
                pt[DF:128, IP:2 * IP], start=False, stop=last)

        outTs = {}

        def ev(p):
            oT = outT_pool.tile([DF + 1, IP], F32, tag="ot")
            # DVE may read only one PSUM operand per instruction
            nc.vector.tensor_copy(oT[:], cur["A"][:])
            nc.vector.tensor_add(oT[:], oT[:], cur["B"][:])
            outTs[p] = oT

        def fin(p):
            oT = outTs[p]
            ob = ob_pool.tile([128, IP // 128, DF], F32, tag="ob")
            for c in range(IP // 128):
                pf = wpsum.tile([128, DF + 1], F32, tag="ps")
                nc.tensor.transpose(
                    pf[:], oT[:, c * 128:(c + 1) * 128],
                    identf[0:DF + 1, 0:DF + 1])
                rcp = rcp_pool.tile([128, 1], F32, tag="rcp")
                nc.vector.reciprocal(rcp[:], pf[:, DF:DF + 1])
                nc.vector.tensor_scalar_mul(ob[:, c, :], pf[:, 0:DF], rcp[:])
            nc.sync.dma_start(
                out[p * IP:(p + 1) * IP, :].rearrange("(c p) f -> p c f", p=128),
                ob[:])

        # ---- emission schedule ----
        # Score tiles stream in TP_ORDER at ~ACT pace (ss ring bufs=2
        # backpressure). A work queue of attn pieces / v-projections /
        # pass evictions / finals is drained between score emissions,
        # gated on (a) the piece's exp tile being >= LAG tiles back,
        # (b) v-projection items waiting for their DMA-arrival slot,
        # (c) pass p waiting for ev(p-1). This keeps the in-order PE
        # queue free of head-of-line blocking.
        sc_order = [(t, p, c) for (t, p) in TP_ORDER for c in range(4)]
        sc_pos = {tpc: i for i, tpc in enumerate(sc_order)}
        LAG = 2

        # work items, in required execution order
        work = []
        for p in range(NP):
            for t in range(NT):
                if p == 0:
                    work.append(("vp", t))
                for c in range(4):
                    work.append(("at", p, t, c))
            work.append(("ev", p))
            work.append(("fin", p))
        # v pair DMA-arrival expressed as "after score tile #n": v pairs
        # land at ~{36,48,54,60}us; ACT clock ~= 9.6us + n*1.15us.
        vp_gate = {0: 22, 1: 32, 2: 38, 3: 42}

        wi = 0
        emitted_n = 0

        def eligible(item, n_now):
            kind = item[0]
            if kind == "vp":
                return n_now >= vp_gate[item[1]]
            if kind == "at":
                _, p, t, c = item
                return sc_pos[(t, p, c)] + LAG <= n_now
            return True  # ev / fin

        def drain(n_now, budget):
            nonlocal wi
            done = 0
            while wi < len(work) and done < budget and eligible(work[wi], n_now):
                item = work[wi]
                if item[0] == "vp":
                    t = item[1]
                    vtrans(t, kvproj(t, 2))
                elif item[0] == "at":
                    _, p, t, c = item
                    if t == 0 and c == 0:
                        pass_begin()
                    at_piece(p, t, c)
                elif item[0] == "ev":
                    ev(item[1])
                else:
                    fin(item[1])
                wi += 1
                done += 1

        qproj(0)
        kvproj(0, 1)
        for n, (t, p, c) in enumerate(sc_order):
            if t == 0 and c == 0 and p > 0:
                qproj(p)          # i-slice p first used here
            if p == 0 and c == 0 and t > 0:
                kvproj(t, 1)      # k pair t first used here
            sc(t, p, c)
            drain(n, 2)
        # tail: drain everything left
        drain(10 ** 9, 10 ** 9)


_COMPILED = None


def get_compiled():
    global _COMPILED
    if _COMPILED is None:
        nc = bacc.Bacc("TRN2", target_bir_lowering=False, debug=False,
                       enable_asserts=False, num_devices=NCORES)
        with tile.TileContext(nc) as tc:
            build_kernel(tc)
        nc.compile()
        _COMPILED = nc
    return _COMPILED


def _to_block_major(xT):
    """[DM, s_len] -> [128, nblk*MC*NI]: 512-col blocks, m-chunk-major inside."""
    s_len = xT.shape[1]
    nblk = s_len // NI
    return np.ascontiguousarray(
        xT.reshape(MC, 128, nblk, NI).transpose(1, 2, 0, 3).reshape(128, nblk * MC * NI))


def make_in_maps(queries, keys, values, Wq, bq, Wk, bk, Wv, bv):
    queries = np.asarray(queries, dtype=np.float32)
    keys = np.asarray(keys, dtype=np.float32)
    values = np.asarray(values, dtype=np.float32)
    WqT, WkT, WvT = np.asarray(Wq).T, np.asarray(Wk).T, np.asarray(Wv).T
    wT_full = np.concatenate([WqT, WqT, WkT, WvT], axis=1)  # [DM, 256]
    wT_host = np.ascontiguousarray(
        wT_full.reshape(MC, 128, WB).transpose(1, 0, 2).reshape(128, MC * WB)
    ).astype(NP_BF16)
    bias64 = np.stack(
        [np.asarray(bq), np.asarray(bk), np.asarray(bv)], axis=1
    ).astype(np.float32)
    bias_host = np.concatenate([bias64, bias64], axis=0)  # [128, 3]

    in_maps = []
    for c in range(NCORES):
        b, h = c // 2, c % 2
        in_maps.append({
            "xq": _to_block_major(queries[b, h * SQ:(h + 1) * SQ, :].T).astype(NP_BF16),
            "xk": _to_block_major(keys[b].T).astype(NP_BF16),
            "xv": _to_block_major(values[b].T).astype(NP_BF16),
            "wT": wT_host, "bias": bias_host,
        })
    return in_maps


def assemble(results):
    out = np.zeros((B, S, DF), dtype=np.float32)
    for c in range(NCORES):
        b, h = c // 2, c % 2
        out[b, h * SQ:(h + 1) * SQ, :] = results[c]["out"]
    return out


def kernel(**inputs):
    nc = get_compiled()
    in_maps = make_in_maps(**inputs)
    res = run_bass_kernel_spmd(nc, in_maps, core_ids=list(range(NCORES)))
    return assemble(res.results)
